# revision 40
# baseline (speedup 1.0000x reference)
"""Trainium2 Bass kernel for nn_ArcEmbedding (embedding lookup + 3-axis RoPE).

Reference computation (per token t in batch b):
    e = emb_table[id]                       # [768]
    theta = [xn*invf, yn*invf, tn*invf]     # [384], xn = x/max(max_b(x),1) etc
    out[0:384]   = e[0:384]*cos(theta) - e[384:768]*sin(theta)
    out[384:768] = e[384:768]*cos(theta) + e[0:384]*sin(theta)

Kernel strategy (data-parallel over batch, 4 batches per NeuronCore, 8 cores):
  Polar refactor: with e1=e[0:384], e2=e[384:768],
      r_s  = sign(e1)*sqrt(e1^2+e2^2)
      phi0 = atan(e2/e1) in (-pi/2, pi/2)
      psi  = phi0 + theta                  (one 384-col matmul per tile)
      out[0:384]   = r_s * cos(psi) = r_s * sin(-psi + pi/2)
      out[384:768] = r_s * sin(psi)
  The cos comes from the SAME psum as the sin using the ACT engine's
  scale/bias (sin(-x + pi/2)), so the PE only streams 384 psi columns
  plus 384 amplitude columns per 128-token tile.
  The host packs onehot(id) + raw x + raw y + normalized t as ONE
  [67, S] DRAM tensor per batch (single input DMA); the per-batch
  1/max normalization is folded into the x/y angle rows of a tiny
  per-batch copy of the stationary-side table, so nothing bulky sits
  on the normalization dependency chain.
  Tiles are processed in PAIRS with the pair interleaved host-side
  (tile A = even tokens, tile B = odd tokens of a 256-token block) so
  one output DMA writes 3072 contiguous bytes per partition.
  Per pair: 4 matmuls (psi A/B + rg A/B, all psum-bank-aligned), 2 ACT
  Sin ops (sin and cos for both tiles), and ONE DVE multiply that reads
  the f32 amplitudes straight from PSUM (broadcast over the lo/hi dim).
"""

import numpy as np

B, S, H, V = 32, 4096, 768, 64
P = 128
NCORES = 8
BPC = B // NCORES            # batches per core
NT = S // P                  # 128-token tiles per batch
NPAIR = NT // 2              # tile pairs per batch
HALF = H // 2                # 384
DA = HALF // 3               # 128 freqs per axis
KL = V + 3                   # lhsT rows: onehot + x + y + t
ROPE_BASE = 10000.0

_INVF = (1.0 / (ROPE_BASE ** (np.arange(DA, dtype=np.float64) / DA))).astype(
    np.float32
)
_TNORM = (np.arange(S, dtype=np.float64) / (S - 1)).astype(np.float32)
# pair interleave: block of 256 tokens -> [even tokens | odd tokens]
_PERM = np.arange(S).reshape(NPAIR, P, 2).transpose(0, 2, 1).reshape(S)

_COMPILED = {}
LAST_RESULTS = None


def _build_program():
    import concourse.bacc as bacc
    import concourse.mybir as mybir
    import concourse.tile as tile

    f32 = mybir.dt.float32
    bf16 = mybir.dt.bfloat16
    AF = mybir.ActivationFunctionType
    ALU = mybir.AluOpType

    nc = bacc.Bacc("TRN2", target_bir_lowering=False, debug=False)

    pk_d = nc.dram_tensor("pk", [BPC, KL, S], bf16, kind="ExternalInput")
    xymax_d = nc.dram_tensor("xymax", [BPC, P, 2 * NT], bf16, kind="ExternalInput")
    emb_d = nc.dram_tensor("emb", [V, H], f32, kind="ExternalInput")
    rtail_d = nc.dram_tensor("rhs_tail", [3, HALF], bf16, kind="ExternalInput")
    ident_d = nc.dram_tensor("ident", [P, P], f32, kind="ExternalInput")
    out_d = nc.dram_tensor("out", [BPC, S, H], bf16, kind="ExternalOutput")

    with tile.TileContext(nc) as tc:
        with (
            tc.tile_pool(name="const", bufs=1) as cpool,
            tc.tile_pool(name="batch", bufs=BPC) as bpool,
            tc.tile_pool(name="work", bufs=4) as wpool,
            tc.tile_pool(name="psum", bufs=2, space="PSUM") as ppool,
        ):
            # ---------------- input DMAs (Sync program order) -------------
            emb_sb = cpool.tile([V, H], f32)
            nc.sync.dma_start(out=emb_sb[:], in_=emb_d[:])
            ident_t = cpool.tile([P, P], f32)
            nc.sync.dma_start(out=ident_t[:], in_=ident_d[:])
            mxins, Ls = [], []
            for b in range(BPC):
                mxin = bpool.tile([P, 2 * NT], bf16, tag="mxin", name=f"mxin{b}")
                nc.sync.dma_start(out=mxin[:], in_=xymax_d[b])
                mxins.append(mxin)
            for b in range(BPC):
                L = bpool.tile([KL, S], bf16, tag="bigL", name=f"L{b}")
                Ls.append(L)
            # only batch 0 loads up front: a scheduler stage barrier right
            # after the prologue waits for ALL in-flight DMAs, so the other
            # batches' big loads are issued from inside the pair loop.
            # split 64+3: a 64-row DMA spreads across the DMA engines,
            # a 67-row one degrades to a single-engine chain
            nc.sync.dma_start(out=Ls[0][0:V, :], in_=pk_d[0, 0:V])
            nc.sync.dma_start(out=Ls[0][V:KL, :], in_=pk_d[0, V:KL])

            # ---------------- one-time table prep ----------------
            # shared table: [psi cols 0:384 | r cols 384:768]
            # rows 0:64 = phi / r_s gather rows, rows 64:67 = angle rows
            # (unscaled; per-batch scaled copies of the psi half below)
            rhs_t = cpool.tile([KL, H], bf16)
            nc.vector.memset(rhs_t[:], 0.0)
            nc.sync.dma_start(out=rhs_t[64:KL, 0:HALF], in_=rtail_d[:])
            halfpi = cpool.tile([P, 1], f32)
            nc.vector.memset(halfpi[:], float(np.pi / 2))

            # phi branch first: it gates the per-batch psi tables
            e1 = emb_sb[:, 0:HALF]
            e2 = emb_sb[:, HALF:H]
            einv = cpool.tile([V, HALF], f32)
            nc.vector.reciprocal_approx_fast(out=einv[:], in_=e1)
            quo = cpool.tile([V, HALF], f32)
            nc.vector.tensor_tensor(out=quo[:], in0=e2, in1=einv[:], op=ALU.mult)
            phi = cpool.tile([V, HALF], f32)
            nc.scalar.activation(out=phi[:], in_=quo[:], func=AF.Arctan)
            nc.scalar.copy(out=rhs_t[0:V, 0:HALF], in_=phi[:])
            # tiny dummy Sin pulls the trig ACT table load off the critical
            # path (it would otherwise happen right before the first pair)
            dummy = cpool.tile([P, 1], f32)
            nc.scalar.activation(out=dummy[:], in_=halfpi[:], func=AF.Sin)

            # r_s = sign(e1)*sqrt(e1^2+e2^2) = e1*sqrt(1+q^2), reusing q from
            # the phi branch; the +1 rides ACT Sqrt's bias port
            q2 = cpool.tile([V, HALF], f32)
            nc.vector.tensor_tensor(out=q2[:], in0=quo[:], in1=quo[:], op=ALU.mult)
            rmag = cpool.tile([V, HALF], f32)
            nc.scalar.activation(out=rmag[:], in_=q2[:], func=AF.Sqrt, bias=1.0)
            rsg = cpool.tile([V, HALF], f32)
            nc.vector.tensor_tensor(out=rsg[:], in0=rmag[:], in1=e1, op=ALU.mult)
            nc.vector.tensor_copy(out=rhs_t[0:V, HALF:H], in_=rsg[:])

            # ---------------- per-batch normalization ----------------
            # max over batch -> 1/max folded into the x/y angle rows of a
            # small per-batch copy of the psi table half
            rhsbs = []
            for b in range(BPC):
                mxin = mxins[b]
                mx2 = bpool.tile([P, 2], f32, tag="mx2", name=f"mx2{b}")
                nc.vector.tensor_reduce(
                    out=mx2[:, 0:1], in_=mxin[:, 0:NT],
                    axis=mybir.AxisListType.X, op=ALU.max,
                )
                nc.vector.tensor_reduce(
                    out=mx2[:, 1:2], in_=mxin[:, NT:2 * NT],
                    axis=mybir.AxisListType.X, op=ALU.max,
                )
                # shares the rg psum slots (PSUM budget: 2*2 + 2*2 = 8 banks)
                pmx = ppool.tile([2, P], f32, tag="rg", name=f"pmx{b}")
                nc.tensor.transpose(out=pmx[:], in_=mx2[:], identity=ident_t[:])
                stg = bpool.tile([2, 4], f32, tag="stg", name=f"stg{b}")
                nc.vector.tensor_reduce(
                    out=stg[:, 0:1], in_=pmx[:],
                    axis=mybir.AxisListType.X, op=ALU.max,
                )
                nc.vector.tensor_scalar(
                    out=stg[:, 1:2], in0=stg[:, 0:1], scalar1=1.0,
                    scalar2=None, op0=ALU.max,
                )
                nc.vector.reciprocal(out=stg[:, 2:3], in_=stg[:, 1:2])
                rhsb = bpool.tile([KL, HALF], bf16, tag="rhsb", name=f"rhsb{b}")
                if b == 0:
                    # batch 0 gates the first matmul: build straight from phi
                    # on DVE, skipping the phi -> rhs_t -> rhsb round-trip
                    nc.vector.tensor_copy(out=rhsb[0:V, :], in_=phi[:])
                    nc.vector.tensor_copy(
                        out=rhsb[V:KL, :], in_=rhs_t[V:KL, 0:HALF]
                    )
                else:
                    nc.vector.tensor_copy(out=rhsb[:], in_=rhs_t[:, 0:HALF])
                nc.vector.tensor_scalar(
                    out=rhsb[64:66, :], in0=rhs_t[64:66, 0:HALF],
                    scalar1=stg[:, 2:3], scalar2=None, op0=ALU.mult,
                )
                rhsbs.append(rhsb)

            # ---------------- main loop: pure pair work ----------------
            for b in range(BPC):
                L = Ls[b]
                rhsb = rhsbs[b]
                for jp in range(NPAIR):
                    if jp == 4 and b + 1 < BPC:
                        nc.sync.dma_start(
                            out=Ls[b + 1][0:V, :], in_=pk_d[b + 1, 0:V]
                        )
                        nc.sync.dma_start(
                            out=Ls[b + 1][V:KL, :], in_=pk_d[b + 1, V:KL]
                        )
                    w0 = jp * 2 * P           # pair start token (natural)
                    LA = L[:, w0:w0 + P]      # even tokens of the block
                    LB = L[:, w0 + P:w0 + 2 * P]  # odd tokens

                    # psi pair: [psiA @0:384 | pad | psiB @512:896 | pad]
                    psi = ppool.tile([P, 1024], f32, tag="psi")
                    nc.tensor.matmul(
                        psi[:, 0:HALF], LA, rhsb[:], start=True, stop=True,
                    )
                    nc.tensor.matmul(
                        psi[:, 512:512 + HALF], LB, rhsb[:], start=True,
                        stop=True,
                    )
                    # rg pair: same layout, r_s gather columns
                    rg = ppool.tile([P, 1024], f32, tag="rg")
                    nc.tensor.matmul(
                        rg[:, 0:HALF], LA, rhs_t[:, HALF:H],
                        start=True, stop=True,
                    )
                    nc.tensor.matmul(
                        rg[:, 512:512 + HALF], LB, rhs_t[:, HALF:H],
                        start=True, stop=True,
                    )

                    psiv = psi[:].rearrange("p (b h) -> p b h", b=2, h=512)[
                        :, :, 0:HALF
                    ]
                    # one trig tile, laid out to line up with ot:
                    # tg = [cosA | sinA | cosB | sinB] (cos = sin(-x+pi/2))
                    tg = wpool.tile([P, 2 * H], bf16, tag="tg", bufs=6)
                    tg4 = tg[:].rearrange("p (b two h) -> p b two h", b=2, two=2)
                    nc.scalar.activation(
                        out=tg4[:, :, 1, :], in_=psiv, func=AF.Sin,
                    )
                    nc.scalar.activation(
                        out=tg4[:, :, 0, :], in_=psiv, func=AF.Sin,
                        scale=-1.0, bias=halfpi[:],
                    )

                    # ot = [loA | hiA | loB | hiB], 384 cols each ->
                    # partition row = [token-even out | token-odd out]
                    # ONE DVE multiply for the whole pair: amplitudes are read
                    # straight from PSUM (broadcast across the lo/hi dim), so
                    # no cast instruction is needed anywhere.
                    rgv = rg[:].rearrange(
                        "p (b one h) -> p b one h", b=2, one=1, h=512
                    )[:, :, :, 0:HALF].to_broadcast([P, 2, 2, HALF])
                    ot = wpool.tile([P, 2 * H], bf16, tag="ot", bufs=8)
                    ot4 = ot[:].rearrange("p (b two h) -> p b two h", b=2, two=2)
                    nc.vector.tensor_tensor(
                        out=ot4[:], in0=rgv, in1=tg4[:], op=ALU.mult,
                    )
                    nc.sync.dma_start(
                        out=out_d[b, w0:w0 + 2 * P, :].rearrange(
                            "(p k) h -> p (k h)", k=2
                        ),
                        in_=ot[:],
                    )

    nc.compile()
    return nc


def _host_inputs(input_ids, coords, emb_table):
    import ml_dtypes

    bf16 = ml_dtypes.bfloat16
    ids = np.asarray(input_ids).astype(np.float32)[:, _PERM]     # [B, S]
    xy = np.asarray(coords).astype(np.float32)[:, _PERM, :]      # [B, S, 2]
    emb = np.asarray(emb_table).astype(np.float32)               # [V, H]
    tnorm = _TNORM[_PERM]

    ident = np.eye(P, dtype=np.float32)
    rtail = np.zeros((3, HALF), dtype=np.float32)
    rtail[0, 0:DA] = _INVF                                   # x angle row
    rtail[1, DA:2 * DA] = _INVF                              # y angle row
    rtail[2, 2 * DA:HALF] = _INVF                            # t angle row
    rtail = rtail.astype(bf16)

    in_maps = []
    for c in range(NCORES):
        bs = slice(c * BPC, (c + 1) * BPC)
        pk = np.empty((BPC, KL, S), dtype=np.float32)
        pk[:, 0:V, :] = (
            ids[bs][:, None, :] == np.arange(V, dtype=np.float32)[None, :, None]
        )
        pk[:, V + 0, :] = xy[bs, :, 0]
        pk[:, V + 1, :] = xy[bs, :, 1]
        pk[:, V + 2, :] = tnorm[None, :]
        xymax = np.empty((BPC, P, 2 * NT), dtype=np.float32)
        xymax[:, :, 0:NT] = xy[bs, :, 0].reshape(BPC, NT, P).transpose(0, 2, 1)
        xymax[:, :, NT:2 * NT] = (
            xy[bs, :, 1].reshape(BPC, NT, P).transpose(0, 2, 1)
        )
        in_maps.append(
            {
                "pk": pk.astype(bf16),
                "xymax": xymax.astype(bf16),
                "emb": emb,
                "rhs_tail": rtail,
                "ident": ident,
            }
        )
    return in_maps


def kernel(input_ids, coords, emb_table):
    global LAST_RESULTS
    from concourse.bass_utils import run_bass_kernel_spmd

    if "nc" not in _COMPILED:
        _COMPILED["nc"] = _build_program()
    nc = _COMPILED["nc"]

    in_maps = _host_inputs(input_ids, coords, emb_table)
    res = run_bass_kernel_spmd(nc, in_maps, core_ids=list(range(NCORES)))
    LAST_RESULTS = res
    out = np.concatenate(
        [r["out"].astype(np.float32) for r in res.results], axis=0
    )
    return out
